# revision 11
# baseline (speedup 1.0000x reference)
"""Trainium2 Bass kernel for CustomLSTM: B=64, T=1024, I=H=512.

Sharding: data-parallel over batch, 8 sequences per core on 8 cores.
Transposed on-device layout throughout (gate/hidden dims on SBUF partitions,
(tile, batch) on the free dim) so elementwise runs on all 128 lanes and h^T
feeds the next step's matmul directly with zero transposes.

v2 structure (single fused pass, no DRAM xw round-trip):
- Phase-1 (x@W+bias) is computed chunk-by-chunk (32 steps) into SBUF in bf16,
  interleaved into the recurrence's PE idle windows (one W m-tile per 2 steps).
- Per step, xw_t is injected into PSUM via identity matmuls (start=True), then
  the 64 U.h matmuls accumulate on top (start=False). This removes the
  per-step DVE ADD from the critical chain; sigmois/tanh read PSUM directly.
- h is written once per step as bf16 into a 33-slot stage buffer; the next
  step's matmuls use the previous slot as the moving operand (no copy/cast).
  Macro boundaries chain by reading the previous stage tile's last slot.
- m-tile order i,f,g,o: the c-path activations/muls overlap the o matmuls.
"""

import numpy as np
import ml_dtypes

B, T, I, H = 64, 1024, 512, 512
NC = 8            # cores
BL = B // NC      # 8 sequences per core
G4 = 4 * H        # 2048 gate dim
KT = I // 128     # 4 contraction tiles (and hidden quarters)
MT = G4 // 128    # 16 gate m-tiles: 0-3=i, 4-7=f, 8-11=g, 12-15=o
MACRO = 32        # timesteps per macro block
NMAC = T // MACRO # 32 macro blocks
XROWS = T + 2 * MACRO  # xT2 padded rows (prefetch overrun)


def build(nc, bass, tile, mybir):
    f32, bf16 = mybir.dt.float32, mybir.dt.bfloat16
    AF = mybir.ActivationFunctionType

    xT2 = nc.dram_tensor("xT2", [128, XROWS, KT * BL], bf16, kind="ExternalInput")
    W = nc.dram_tensor("W", [128, KT, G4], bf16, kind="ExternalInput")
    U = nc.dram_tensor("U", [128, KT, G4], bf16, kind="ExternalInput")
    biasT = nc.dram_tensor("biasT", [128, MT], f32, kind="ExternalInput")
    eye = nc.dram_tensor("eye", [128, 128], bf16, kind="ExternalInput")
    hT_out = nc.dram_tensor("hT_out", [128, T, KT * BL], bf16, kind="ExternalOutput")

    SL = KT * BL  # 32: (k, b) columns of h / c state

    with tile.TileContext(nc) as tc:
        with (
            tc.tile_pool(name="const", bufs=1) as const,
            tc.tile_pool(name="pre_ps", bufs=2, space="PSUM") as pre_ps,
            tc.tile_pool(name="g_ps", bufs=2, space="PSUM") as g_ps,
            tc.tile_pool(name="work", bufs=2) as work,
        ):
            W_sb = const.tile([128, KT, G4], bf16)
            U_sb = const.tile([128, KT, G4], bf16)
            bias_sb = const.tile([128, MT], f32)
            eye_sb = const.tile([128, 128], bf16)
            c_st = const.tile([128, SL], f32)
            h0 = const.tile([128, SL], bf16)
            # Explicit ping-pong buffers for cross-macro pipelines (persistent
            # tiles -> fixed addresses, required inside the hardware loop).
            chunks = [const.tile([128, MACRO, 128], bf16, name=f"chunk{i}")
                      for i in range(2)]
            xtcs = [const.tile([128, KT, MACRO * BL], bf16, name=f"xtc{i}")
                    for i in range(2)]
            stages = [const.tile([128, MACRO + 1, SL], bf16, name=f"stg{i}")
                      for i in range(2)]
            nc.gpsimd.dma_start(W_sb[:], W[:])
            nc.gpsimd.dma_start(U_sb[:], U[:])
            nc.gpsimd.dma_start(bias_sb[:], biasT[:])
            nc.gpsimd.dma_start(eye_sb[:], eye[:])
            nc.vector.memset(c_st[:], 0.0)
            nc.vector.memset(h0[:], 0.0)

            def fetch_x(row0, xtc):
                """DMA one chunk of x rows into a k-major xtc tile."""
                for k in range(KT):
                    nc.sync.dma_start(
                        xtc[:, k, :].rearrange("p (t b) -> p t b", t=MACRO),
                        xT2[:, bass.ds(row0, MACRO), k * BL:(k + 1) * BL],
                    )

            def phase1_mtile(xtc, chunk, m):
                """xw for one W m-tile over a 32-step chunk -> chunk SBUF."""
                # Full-bank tile: PSUM hazards are bank-granular; sharing a
                # bank with the per-step gate tiles serializes the pipeline.
                ps = pre_ps.tile([128, 512], f32, name="pps")[:, 0:MACRO * BL]
                for k in range(KT):
                    nc.tensor.matmul(
                        ps[:],
                        W_sb[:, k, m * 128:(m + 1) * 128],
                        xtc[:, k, :],
                        start=(k == 0),
                        stop=(k == KT - 1),
                    )
                nc.scalar.activation(
                    chunk[:, :, m * BL:(m + 1) * BL],
                    ps[:].rearrange("p (t b) -> p t b", t=MACRO),
                    AF.Identity,
                    bias=bias_sb[:, m:m + 1],
                )

            def step(chunk, stage, prev_h, s):
                """One recurrence step; h input = prev_h AP, h out -> stage."""
                ps_if = g_ps.tile([128, 512], f32, tag="if", name="psif")[:, 0:8 * BL]
                ps_g = g_ps.tile([128, 512], f32, tag="g", name="psg")[:, 0:4 * BL]
                ps_o = g_ps.tile([128, 512], f32, tag="o", name="pso")[:, 0:4 * BL]
                # xw injection (no h dependency; runs during previous tail)
                nc.tensor.matmul(ps_if[:], eye_sb[:], chunk[:, s, 0:64],
                                 start=True, stop=False)
                nc.tensor.matmul(ps_g[:], eye_sb[:], chunk[:, s, 64:96],
                                 start=True, stop=False)
                nc.tensor.matmul(ps_o[:], eye_sb[:], chunk[:, s, 96:128],
                                 start=True, stop=False)
                # U.h accumulation, m-tile order i,f,g,o
                for m in range(MT):
                    if m < 8:
                        dst = ps_if[:, m * BL:(m + 1) * BL]
                    elif m < 12:
                        dst = ps_g[:, (m - 8) * BL:(m - 7) * BL]
                    else:
                        dst = ps_o[:, (m - 12) * BL:(m - 11) * BL]
                    for k in range(KT):
                        nc.tensor.matmul(
                            dst,
                            U_sb[:, k, m * 128:(m + 1) * 128],
                            prev_h[:, k * BL:(k + 1) * BL],
                            start=False,
                            stop=(k == KT - 1),
                        )
                act_if = work.tile([128, 8 * BL], f32, tag="aif")
                act_g = work.tile([128, 4 * BL], f32, tag="ag")
                act_o = work.tile([128, 4 * BL], f32, tag="ao")
                nc.scalar.activation(act_if[:], ps_if[:], AF.Sigmoid)
                nc.scalar.activation(act_g[:], ps_g[:], AF.Tanh)
                nc.scalar.activation(act_o[:], ps_o[:], AF.Sigmoid)
                fc = work.tile([128, SL], f32, tag="fc")
                ig = work.tile([128, SL], f32, tag="ig")
                nc.vector.tensor_mul(fc[:], act_if[:, SL:2 * SL], c_st[:])
                nc.vector.tensor_mul(ig[:], act_if[:, 0:SL], act_g[:])
                nc.vector.tensor_add(c_st[:], fc[:], ig[:])
                tc_t = work.tile([128, SL], f32, tag="tc")
                nc.scalar.activation(tc_t[:], c_st[:], AF.Tanh)
                nc.vector.tensor_mul(stage[:, s + 1, :], act_o[:], tc_t[:])

            def emit_macro(iv, par, first=False, last=False):
                """One macro: steps on chunks[par], phase-1 for the next chunk
                into chunks[1-par] (reading xtcs[1-par]), prefetch x for the
                chunk after that into xtcs[par]."""
                chunk = chunks[par]
                stage = stages[par]
                prev_stage = stages[1 - par]
                if not last:
                    fetch_x(iv + 2 * MACRO, xtcs[par])
                for s in range(MACRO):
                    if s == 0:
                        prev_h = h0[:] if first else prev_stage[:, MACRO, :]
                    else:
                        prev_h = stage[:, s, :]
                    step(chunk, stage, prev_h, s)
                    if (not last) and s % 2 == 0:
                        phase1_mtile(xtcs[1 - par], chunks[1 - par], s // 2)
                nc.sync.dma_start(
                    hT_out[:, bass.ds(iv, MACRO), :],
                    stage[:, 1:MACRO + 1, :],
                )

            # Prologue: x rows 0:32 and 32:64, then chunk 0 serially.
            fetch_x(0, xtcs[0])
            fetch_x(MACRO, xtcs[1])
            for m in range(MT):
                phase1_mtile(xtcs[0], chunks[0], m)

            emit_macro(0, 0, first=True)

            def loop_body(iv, unroll):
                for u in range(unroll):
                    emit_macro(iv + u * MACRO, (1 + u) % 2)

            tc.For_i_unrolled_general(
                start=MACRO, end=(NMAC - 1) * MACRO, step=MACRO,
                unrollable_body=loop_body, max_unroll=2,
                hint_engines=(mybir.EngineType.PE,),
            )

            emit_macro((NMAC - 1) * MACRO, (NMAC - 1) % 2, last=True)
    nc.finalize()
    return nc


def kernel(x, W, U, bias):
    import concourse.bass as bass
    import concourse.bacc as bacc
    import concourse.tile as tile
    import concourse.mybir as mybir
    from concourse.bass_utils import run_bass_kernel_spmd

    x = np.asarray(x, np.float32)
    W = np.asarray(W, np.float32)
    U = np.asarray(U, np.float32)
    bias = np.asarray(bias, np.float32)

    nc = build(bacc.Bacc("TRN2", target_bir_lowering=False, num_devices=NC),
               bass, tile, mybir)

    Wt = np.ascontiguousarray(
        W.reshape(KT, 128, G4).transpose(1, 0, 2)
    ).astype(ml_dtypes.bfloat16)
    Ut = np.ascontiguousarray(
        U.reshape(KT, 128, G4).transpose(1, 0, 2)
    ).astype(ml_dtypes.bfloat16)
    bt = np.ascontiguousarray(bias.reshape(MT, 128).T)
    ey = np.eye(128, dtype=np.float32).astype(ml_dtypes.bfloat16)

    in_maps = []
    for i in range(NC):
        xl = x[i * BL:(i + 1) * BL]                      # [8, 1024, 512]
        # xT2[p, t, k*8+b] = x[b, t, k*128+p]
        xt = xl.reshape(BL, T, KT, 128).transpose(3, 1, 2, 0).reshape(128, T, KT * BL)
        xp = np.zeros((128, XROWS, KT * BL), np.float32)
        xp[:, :T] = xt
        in_maps.append({
            "xT2": xp.astype(ml_dtypes.bfloat16),
            "W": Wt, "U": Ut, "biasT": bt, "eye": ey,
        })

    import os
    trace = bool(os.environ.get("LSTM_TRACE"))
    res = run_bass_kernel_spmd(
        nc, in_maps, core_ids=list(range(NC)), trace=trace
    )
    if trace and res.exec_time_ns is not None:
        print(f"HW exec time: {res.exec_time_ns} ns")
        print("trace:", (res.instructions_and_trace or (None, None))[1])
    out = np.empty((B, T, H), np.float32)
    for i in range(NC):
        ho = np.asarray(res.results[i]["hT_out"], dtype=np.float32)  # [128,1024,32]
        # out[b, t, k*128+p] = ho[p, t, k*8+b]
        out[i * BL:(i + 1) * BL] = (
            ho.reshape(128, T, KT, BL).transpose(3, 1, 2, 0).reshape(BL, T, H)
        )
    return out


# revision 12
# speedup vs baseline: 1.0002x; 1.0002x over previous
"""Trainium2 Bass kernel for CustomLSTM: B=64, T=1024, I=H=512.

Sharding: data-parallel over batch, 8 sequences per core on 8 cores.
Transposed on-device layout throughout (gate/hidden dims on SBUF partitions,
(tile, batch) on the free dim) so elementwise runs on all 128 lanes and h^T
feeds the next step's matmul directly with zero transposes.

v2 structure (single fused pass, no DRAM xw round-trip):
- Phase-1 (x@W+bias) is computed chunk-by-chunk (32 steps) into SBUF in bf16,
  interleaved into the recurrence's PE idle windows (one W m-tile per 2 steps).
- Per step, xw_t is injected into PSUM via identity matmuls (start=True), then
  the 64 U.h matmuls accumulate on top (start=False). This removes the
  per-step DVE ADD from the critical chain; sigmois/tanh read PSUM directly.
- h is written once per step as bf16 into a 33-slot stage buffer; the next
  step's matmuls use the previous slot as the moving operand (no copy/cast).
  Macro boundaries chain by reading the previous stage tile's last slot.
- m-tile order i,f,g,o: the c-path activations/muls overlap the o matmuls.
"""

import numpy as np
import ml_dtypes

B, T, I, H = 64, 1024, 512, 512
NC = 8            # cores
BL = B // NC      # 8 sequences per core
G4 = 4 * H        # 2048 gate dim
KT = I // 128     # 4 contraction tiles (and hidden quarters)
MT = G4 // 128    # 16 gate m-tiles: 0-3=i, 4-7=f, 8-11=g, 12-15=o
MACRO = 32        # timesteps per macro block
NMAC = T // MACRO # 32 macro blocks
XROWS = T + 2 * MACRO  # xT2 padded rows (prefetch overrun)


def build(nc, bass, tile, mybir):
    f32, bf16 = mybir.dt.float32, mybir.dt.bfloat16
    AF = mybir.ActivationFunctionType

    xT2 = nc.dram_tensor("xT2", [128, XROWS, KT * BL], bf16, kind="ExternalInput")
    W = nc.dram_tensor("W", [128, KT, G4], bf16, kind="ExternalInput")
    U = nc.dram_tensor("U", [128, KT, G4], bf16, kind="ExternalInput")
    biasT = nc.dram_tensor("biasT", [128, MT], f32, kind="ExternalInput")
    eye = nc.dram_tensor("eye", [128, 128], bf16, kind="ExternalInput")
    hT_out = nc.dram_tensor("hT_out", [128, T, KT * BL], bf16, kind="ExternalOutput")

    SL = KT * BL  # 32: (k, b) columns of h / c state

    with tile.TileContext(nc) as tc:
        with (
            tc.tile_pool(name="const", bufs=1) as const,
            tc.tile_pool(name="pre_ps", bufs=2, space="PSUM") as pre_ps,
            tc.tile_pool(name="g_ps", bufs=2, space="PSUM") as g_ps,
            tc.tile_pool(name="work", bufs=2) as work,
        ):
            W_sb = const.tile([128, KT, G4], bf16)
            U_sb = const.tile([128, KT, G4], bf16)
            bias_sb = const.tile([128, MT], f32)
            eye_sb = const.tile([128, 128], bf16)
            c_st = const.tile([128, SL], f32)
            h0 = const.tile([128, SL], bf16)
            # Explicit ping-pong buffers for cross-macro pipelines (persistent
            # tiles -> fixed addresses, required inside the hardware loop).
            chunks = [const.tile([128, MACRO, 128], bf16, name=f"chunk{i}")
                      for i in range(2)]
            xtcs = [const.tile([128, KT, MACRO * BL], bf16, name=f"xtc{i}")
                    for i in range(2)]
            stages = [const.tile([128, MACRO + 1, SL], bf16, name=f"stg{i}")
                      for i in range(2)]
            nc.gpsimd.dma_start(W_sb[:], W[:])
            nc.gpsimd.dma_start(U_sb[:], U[:])
            nc.gpsimd.dma_start(bias_sb[:], biasT[:])
            nc.gpsimd.dma_start(eye_sb[:], eye[:])
            nc.vector.memset(c_st[:], 0.0)
            nc.vector.memset(h0[:], 0.0)

            def fetch_x(row0, xtc):
                """DMA one chunk of x rows into a k-major xtc tile."""
                for k in range(KT):
                    nc.sync.dma_start(
                        xtc[:, k, :].rearrange("p (t b) -> p t b", t=MACRO),
                        xT2[:, bass.ds(row0, MACRO), k * BL:(k + 1) * BL],
                    )

            def phase1_mtile(xtc, chunk, m):
                """xw for one W m-tile over a 32-step chunk -> chunk SBUF."""
                # Full-bank tile: PSUM hazards are bank-granular; sharing a
                # bank with the per-step gate tiles serializes the pipeline.
                ps = pre_ps.tile([128, 512], f32, name="pps")[:, 0:MACRO * BL]
                for k in range(KT):
                    nc.tensor.matmul(
                        ps[:],
                        W_sb[:, k, m * 128:(m + 1) * 128],
                        xtc[:, k, :],
                        start=(k == 0),
                        stop=(k == KT - 1),
                    )
                nc.scalar.activation(
                    chunk[:, :, m * BL:(m + 1) * BL],
                    ps[:].rearrange("p (t b) -> p t b", t=MACRO),
                    AF.Identity,
                    bias=bias_sb[:, m:m + 1],
                )

            def step(chunk, stage, prev_h, s):
                """One recurrence step; h input = prev_h AP, h out -> stage."""
                ps_if = g_ps.tile([128, 512], f32, tag="if", name="psif")[:, 0:8 * BL]
                ps_g = g_ps.tile([128, 512], f32, tag="g", name="psg")[:, 0:4 * BL]
                ps_o = g_ps.tile([128, 512], f32, tag="o", name="pso")[:, 0:4 * BL]
                # xw injection (no h dependency; runs during previous tail)
                nc.tensor.matmul(ps_if[:], eye_sb[:], chunk[:, s, 0:64],
                                 start=True, stop=False)
                nc.tensor.matmul(ps_g[:], eye_sb[:], chunk[:, s, 64:96],
                                 start=True, stop=False)
                nc.tensor.matmul(ps_o[:], eye_sb[:], chunk[:, s, 96:128],
                                 start=True, stop=False)
                # U.h accumulation, m-tile order i,f,g,o
                for m in range(MT):
                    if m < 8:
                        dst = ps_if[:, m * BL:(m + 1) * BL]
                    elif m < 12:
                        dst = ps_g[:, (m - 8) * BL:(m - 7) * BL]
                    else:
                        dst = ps_o[:, (m - 12) * BL:(m - 11) * BL]
                    for k in range(KT):
                        nc.tensor.matmul(
                            dst,
                            U_sb[:, k, m * 128:(m + 1) * 128],
                            prev_h[:, k * BL:(k + 1) * BL],
                            start=False,
                            stop=(k == KT - 1),
                        )
                act_if = work.tile([128, 8 * BL], f32, tag="aif")
                act_g = work.tile([128, 4 * BL], f32, tag="ag")
                act_o = work.tile([128, 4 * BL], f32, tag="ao")
                nc.scalar.activation(act_if[:], ps_if[:], AF.Sigmoid)
                nc.scalar.activation(act_g[:], ps_g[:], AF.Tanh)
                nc.scalar.activation(act_o[:], ps_o[:], AF.Sigmoid)
                fc = work.tile([128, SL], f32, tag="fc")
                ig = work.tile([128, SL], f32, tag="ig")
                nc.vector.tensor_mul(fc[:], act_if[:, SL:2 * SL], c_st[:])
                nc.vector.tensor_mul(ig[:], act_if[:, 0:SL], act_g[:])
                nc.vector.tensor_add(c_st[:], fc[:], ig[:])
                tc_t = work.tile([128, SL], f32, tag="tc")
                nc.scalar.activation(tc_t[:], c_st[:], AF.Tanh)
                nc.vector.tensor_mul(stage[:, s + 1, :], act_o[:], tc_t[:])

            def emit_macro(iv, par, first=False, last=False):
                """One macro: steps on chunks[par], phase-1 for the next chunk
                into chunks[1-par] (reading xtcs[1-par]), prefetch x for the
                chunk after that into xtcs[par]."""
                chunk = chunks[par]
                stage = stages[par]
                prev_stage = stages[1 - par]
                if not last:
                    fetch_x(iv + 2 * MACRO, xtcs[par])
                for s in range(MACRO):
                    if s == 0:
                        prev_h = h0[:] if first else prev_stage[:, MACRO, :]
                    else:
                        prev_h = stage[:, s, :]
                    step(chunk, stage, prev_h, s)
                    if (not last) and s % 2 == 0:
                        phase1_mtile(xtcs[1 - par], chunks[1 - par], s // 2)
                nc.sync.dma_start(
                    hT_out[:, bass.ds(iv, MACRO), :],
                    stage[:, 1:MACRO + 1, :],
                )

            # Prologue: x rows 0:32 and 32:64, then chunk 0 serially.
            fetch_x(0, xtcs[0])
            fetch_x(MACRO, xtcs[1])
            for m in range(MT):
                phase1_mtile(xtcs[0], chunks[0], m)

            emit_macro(0, 0, first=True)

            with tc.For_i(
                MACRO, (NMAC - 2) * MACRO, 2 * MACRO,
                hint_engines=(mybir.EngineType.PE,), staggered_reset=True,
            ) as iv:
                emit_macro(iv, 1)
                emit_macro(iv + MACRO, 0)

            emit_macro((NMAC - 1) * MACRO, (NMAC - 1) % 2, last=True)
    nc.finalize()
    return nc


def kernel(x, W, U, bias):
    import concourse.bass as bass
    import concourse.bacc as bacc
    import concourse.tile as tile
    import concourse.mybir as mybir
    from concourse.bass_utils import run_bass_kernel_spmd

    x = np.asarray(x, np.float32)
    W = np.asarray(W, np.float32)
    U = np.asarray(U, np.float32)
    bias = np.asarray(bias, np.float32)

    nc = build(bacc.Bacc("TRN2", target_bir_lowering=False, num_devices=NC),
               bass, tile, mybir)

    Wt = np.ascontiguousarray(
        W.reshape(KT, 128, G4).transpose(1, 0, 2)
    ).astype(ml_dtypes.bfloat16)
    Ut = np.ascontiguousarray(
        U.reshape(KT, 128, G4).transpose(1, 0, 2)
    ).astype(ml_dtypes.bfloat16)
    bt = np.ascontiguousarray(bias.reshape(MT, 128).T)
    ey = np.eye(128, dtype=np.float32).astype(ml_dtypes.bfloat16)

    in_maps = []
    for i in range(NC):
        xl = x[i * BL:(i + 1) * BL]                      # [8, 1024, 512]
        # xT2[p, t, k*8+b] = x[b, t, k*128+p]
        xt = xl.reshape(BL, T, KT, 128).transpose(3, 1, 2, 0).reshape(128, T, KT * BL)
        xp = np.zeros((128, XROWS, KT * BL), np.float32)
        xp[:, :T] = xt
        in_maps.append({
            "xT2": xp.astype(ml_dtypes.bfloat16),
            "W": Wt, "U": Ut, "biasT": bt, "eye": ey,
        })

    import os
    trace = bool(os.environ.get("LSTM_TRACE"))
    res = run_bass_kernel_spmd(
        nc, in_maps, core_ids=list(range(NC)), trace=trace
    )
    if trace and res.exec_time_ns is not None:
        print(f"HW exec time: {res.exec_time_ns} ns")
        print("trace:", (res.instructions_and_trace or (None, None))[1])
    out = np.empty((B, T, H), np.float32)
    for i in range(NC):
        ho = np.asarray(res.results[i]["hT_out"], dtype=np.float32)  # [128,1024,32]
        # out[b, t, k*128+p] = ho[p, t, k*8+b]
        out[i * BL:(i + 1) * BL] = (
            ho.reshape(128, T, KT, BL).transpose(3, 1, 2, 0).reshape(BL, T, H)
        )
    return out
